# revision 11
# baseline (speedup 1.0000x reference)
"""Trainium2 Bass kernel for nn_CLF_QP_Net (2-layer tanh Lyapunov MLP + analytic CLF-QP).

Strategy: pure data parallel over 8 NeuronCores (batch 65536 -> 8192/core).
On-chip layout keeps hidden dim on SBUF partitions ([h, b]), batch as the
matmul moving dim (tiles of 512). The two [B,512]x[512,512] matmuls run in
fp16 (1 cyc/row on PE, 10-bit mantissa) with fp32 PSUM accumulation; layer 1
(K=2) uses an error-compensated hi/lo fp16 split folded with the bias into a
single K=10 matmul per output chunk. tanh outputs stay fp32 so the (1-a^2)
tanh' factors avoid catastrophic rounding; only matmul operands are cast to
fp16. h-reductions (V, grad_V) run on TensorE; the per-row QP tail runs once
per core in a [128, 64] layout on VectorE/ScalarE.
"""

import os
import numpy as np

FP16 = np.float16

# Problem constants (hardcoded per contract; kernel.py must be self-contained)
B = 65536
N_IN = 2
H = 512
NCORES = 8
BPC = B // NCORES          # 8192 rows per core
NTILE = 512                # batch tile (matmul moving dim)
NT = BPC // NTILE          # 16 tiles per core
P = 128                    # SBUF partitions
HC = H // P                # 4 hidden chunks
TCOLS = BPC // P           # 64 tail columns

GRAV = 9.81
LENGTH = 1.0
DAMP = 0.1
LOW_M = 1.0
LOW_I = 1.05
CLF_LAMBDA = 1.0
RELAX_PENALTY = 10.0

C_SIN = LOW_M * GRAV * LENGTH / LOW_I
C_DAMP = DAMP / LOW_I
C_QP = 1.0 / (2.0 * RELAX_PENALTY)   # 0.05

last_results = None  # test.py can inspect profile info here
last_nc = None
last_in_maps = None


def _split_fp16(a):
    a = np.asarray(a, np.float32)
    hi = a.astype(FP16)
    lo = (a - hi.astype(np.float32)).astype(FP16)
    return hi, lo


def _build_module():
    import concourse.bass as bass
    import concourse.bacc as bacc
    import concourse.mybir as mybir
    import concourse.tile as tile

    dt = mybir.dt
    Act = mybir.ActivationFunctionType
    Op = mybir.AluOpType

    nc = bacc.Bacc(None, target_bir_lowering=False)

    # --- I/O ---
    xa_d = nc.declare_dram_parameter("xa", [10, BPC], dt.float16, isOutput=False)
    xc_d = nc.declare_dram_parameter("xc", [N_IN, P, TCOLS], dt.float32, isOutput=False)
    w1a_d = nc.declare_dram_parameter("w1a", [10, H], dt.float16, isOutput=False)
    w2f_d = nc.declare_dram_parameter("w2f", [P, HC * H], dt.float16, isOutput=False)
    w2b_d = nc.declare_dram_parameter("w2b", [P, HC * H], dt.float16, isOutput=False)
    w1gv_d = nc.declare_dram_parameter("w1gv", [P, HC * N_IN], dt.float16, isOutput=False)
    b2t_d = nc.declare_dram_parameter("b2t", [P, HC], dt.float32, isOutput=False)
    ho_d = nc.declare_dram_parameter("ho", [P, 1], dt.float16, isOutput=False)
    kk_d = nc.declare_dram_parameter("kk", [P, 2], dt.float32, isOutput=False)

    u_d = nc.declare_dram_parameter("u", [P, TCOLS], dt.float32, isOutput=True)
    r_d = nc.declare_dram_parameter("r", [P, TCOLS], dt.float32, isOutput=True)
    v_d = nc.declare_dram_parameter("v", [NT, NTILE], dt.float32, isOutput=True)
    vd_d = nc.declare_dram_parameter("vd", [P, TCOLS], dt.float32, isOutput=True)

    gv_s = nc.dram_tensor("gv_s", [N_IN, BPC], dt.float32)  # grad_V scratch

    with tile.TileContext(nc) as tc:
        with (
            tc.tile_pool(name="const", bufs=1) as cp,
            tc.tile_pool(name="xin", bufs=3) as xp,
            tc.tile_pool(name="act", bufs=2) as ap,
            tc.tile_pool(name="pz1", bufs=2, space="PSUM") as pz1,
            tc.tile_pool(name="pz2", bufs=2, space="PSUM") as pz2,
            tc.tile_pool(name="pm", bufs=2, space="PSUM") as pm,
            tc.tile_pool(name="psm", bufs=2, space="PSUM") as psm,
            tc.tile_pool(name="tail", bufs=1) as tp,
        ):
            # ---- persistent weights ----
            w1a_t = cp.tile([10, H], dt.float16)
            nc.sync.dma_start(w1a_t[:], w1a_d[:])
            w2f_t = cp.tile([P, HC * H], dt.float16)
            nc.sync.dma_start(w2f_t[:], w2f_d[:])
            w2b_t = cp.tile([P, HC * H], dt.float16)
            nc.sync.dma_start(w2b_t[:], w2b_d[:])
            w1gv_t = cp.tile([P, HC * N_IN], dt.float16)
            nc.sync.dma_start(w1gv_t[:], w1gv_d[:])
            b2t_t = cp.tile([P, HC], dt.float32)
            nc.sync.dma_start(b2t_t[:], b2t_d[:])
            ho_t = cp.tile([P, 1], dt.float16)
            nc.sync.dma_start(ho_t[:], ho_d[:])
            kk_t = cp.tile([P, 2], dt.float32)
            nc.sync.dma_start(kk_t[:], kk_d[:])

            # ---- main batch-tile loop ----
            for t in range(NT):
                xa_t = xp.tile([10, NTILE], dt.float16)
                nc.sync.dma_start(xa_t[:], xa_d[:, t * NTILE:(t + 1) * NTILE])

                # L1: z1 = W1.x + b1 (hi/lo compensated, bias folded), a1 = tanh
                a1 = ap.tile([P, HC, NTILE], dt.float32)
                for mc in range(HC):
                    z1 = pz1.tile([P, NTILE], dt.float32)
                    nc.tensor.matmul(
                        z1[:], w1a_t[:, mc * P:(mc + 1) * P], xa_t[:],
                        start=True, stop=True,
                    )
                    nc.scalar.activation(a1[:, mc, :], z1[:], Act.Tanh)

                a1h = ap.tile([P, HC, NTILE], dt.float16)
                nc.vector.tensor_copy(a1h[:], a1[:])

                # L2 fwd: z2 = W2.a1 + b2, a2 = tanh (fp32 out)
                a2 = ap.tile([P, HC, NTILE], dt.float32)
                for mc in range(HC):
                    z2 = pz2.tile([P, NTILE], dt.float32)
                    for kc in range(HC):
                        nc.tensor.matmul(
                            z2[:],
                            w2f_t[:, kc * H + mc * P: kc * H + (mc + 1) * P],
                            a1h[:, kc, :],
                            start=(kc == 0), stop=(kc == HC - 1),
                        )
                    nc.scalar.activation(
                        a2[:, mc, :], z2[:], Act.Tanh, bias=b2t_t[:, mc:mc + 1]
                    )

                # tanh' algebra: fp32 squares, fp16 matmul operands
                s2 = ap.tile([P, HC, NTILE], dt.float32)
                nc.vector.tensor_tensor(s2[:], a2[:], a2[:], Op.mult)
                s2h = ap.tile([P, HC, NTILE], dt.float16)   # V rhs
                nc.scalar.activation(s2h[:], a2[:], Act.Square)
                t2n = ap.tile([P, HC, NTILE], dt.float16)   # = a2^3 - a2 = -t2
                nc.vector.scalar_tensor_tensor(
                    t2n[:], s2[:], 1.0, a2[:], Op.subtract, Op.mult
                )
                s1 = ap.tile([P, HC, NTILE], dt.float32)
                nc.vector.tensor_tensor(s1[:], a1[:], a1[:], Op.mult)

                # L2 bwd: m' = -(t2 @ W2)^T layout ; t1 = (s1-1)*m' = (1-a1^2)*m
                t1 = ap.tile([P, HC, NTILE], dt.float16)
                for mc in range(HC):
                    m_ = pm.tile([P, NTILE], dt.float32)
                    for kc in range(HC):
                        nc.tensor.matmul(
                            m_[:],
                            w2b_t[:, kc * H + mc * P: kc * H + (mc + 1) * P],
                            t2n[:, kc, :],
                            start=(kc == 0), stop=(kc == HC - 1),
                        )
                    nc.vector.scalar_tensor_tensor(
                        t1[:, mc, :], s1[:, mc, :], 1.0, m_[:],
                        Op.subtract, Op.mult,
                    )

                # V = 0.5*sum_h a2^2 and grad_V = t1^T W1 (partition reductions on PE)
                sm = psm.tile([34, NTILE], dt.float32)
                for kc in range(HC):
                    nc.tensor.matmul(
                        sm[0:1, :], ho_t[:], s2h[:, kc, :],
                        start=(kc == 0), stop=(kc == HC - 1),
                    )
                for kc in range(HC):
                    nc.tensor.matmul(
                        sm[32:34, :], w1gv_t[:, kc * N_IN:(kc + 1) * N_IN],
                        t1[:, kc, :],
                        start=(kc == 0), stop=(kc == HC - 1),
                    )

                vg = xp.tile([34, NTILE], dt.float32, tag="vg")
                nc.vector.tensor_copy(vg[:], sm[:])
                nc.sync.dma_start(v_d[t:t + 1, :], vg[0:1, :])
                nc.sync.dma_start(gv_s[:, t * NTILE:(t + 1) * NTILE], vg[32:34, :])

            # ---- QP tail, whole per-core batch in [128, 64] layout ----
            def tt(name, i0, i1, op):
                o = tp.tile([P, TCOLS], dt.float32, tag=name)
                nc.vector.tensor_tensor(o[:], i0[:], i1[:], op)
                return o

            def stt(name, i0, s, i1, op0, op1):
                o = tp.tile([P, TCOLS], dt.float32, tag=name)
                nc.vector.scalar_tensor_tensor(o[:], i0[:], s, i1[:], op0, op1)
                return o

            def ts(name, i0, s1v, s2v, op0, op1):
                o = tp.tile([P, TCOLS], dt.float32, tag=name)
                if s2v is None:
                    nc.vector.tensor_scalar(o[:], i0[:], s1v, None, op0)
                else:
                    nc.vector.tensor_scalar(o[:], i0[:], s1v, s2v, op0, op1)
                return o

            g0 = tp.tile([P, TCOLS], dt.float32, tag="g0")
            nc.sync.dma_start(g0[:], gv_s[0].rearrange("(p c) -> p c", p=P))
            g1 = tp.tile([P, TCOLS], dt.float32, tag="g1")
            nc.sync.dma_start(g1[:], gv_s[1].rearrange("(p c) -> p c", p=P))
            vt = tp.tile([P, TCOLS], dt.float32, tag="vt")
            nc.sync.dma_start(
                vt[:],
                v_d[:].rearrange("t n -> (t n)").rearrange("(p c) -> p c", p=P),
            )
            x0 = tp.tile([P, TCOLS], dt.float32, tag="x0")
            nc.sync.dma_start(x0[:], xc_d[0])
            x1 = tp.tile([P, TCOLS], dt.float32, tag="x1")
            nc.sync.dma_start(x1[:], xc_d[1])

            sx = tp.tile([P, TCOLS], dt.float32, tag="sx")
            nc.scalar.activation(sx[:], x0[:], Act.Sin)

            # L_f_V = (g0 - c_damp*g1)*x1 + c_sin*g1*sin(x0)
            A = stt("A", g1, C_DAMP, g0, Op.mult, Op.subtract)  # (g1*cd) - g0 = -A
            Bt = tt("Bt", A, x1, Op.mult)                       # -(g0-cd*g1)*x1
            C = stt("C", g1, C_SIN, sx, Op.mult, Op.mult)       # c_sin*g1*sinx
            Lf = stt("Lf", Bt, -1.0, C, Op.mult, Op.add)        # -Bt + C = L_f_V
            Lg = ts("Lg", g1, 1.0 / LOW_I, None, Op.mult, None)

            # u_nom = -(K00*x0 + K01*x1); kk holds -K so no extra negate
            e0 = tp.tile([P, TCOLS], dt.float32, tag="e0")
            nc.vector.tensor_scalar(e0[:], x0[:], kk_t[:, 0:1], None, Op.mult)
            un = tp.tile([P, TCOLS], dt.float32, tag="un")
            nc.vector.scalar_tensor_tensor(
                un[:], x1[:], kk_t[:, 1:2], e0[:], Op.mult, Op.add
            )

            F = tt("F", Lg, un, Op.mult)
            s_ = tt("s_", F, Lf, Op.add)
            s_ = tt("s2_", s_, vt, Op.add)                      # lambda = 1
            G = tt("G", Lg, Lg, Op.mult)
            dn = ts("dn", G, C_QP, None, Op.add, None)
            rec = tp.tile([P, TCOLS], dt.float32, tag="rec")
            scr = tp.tile([P, TCOLS], dt.float32, tag="scr")
            nc.vector.reciprocal_approx_accurate(rec[:], dn[:], scr[:])
            smx = ts("smx", s_, 0.0, None, Op.max, None)
            mu = tt("mu", smx, rec, Op.mult)

            uo = stt("uo", mu, -1.0, Lg, Op.mult, Op.mult)      # -mu*Lg
            uo = tt("uo2", uo, un, Op.add)                      # u = un - mu*Lg
            ro = ts("ro", mu, C_QP, None, Op.mult, None)
            I_ = tt("I_", Lg, uo, Op.mult)
            vd = tt("vd", I_, Lf, Op.add)

            nc.sync.dma_start(u_d[:], uo[:])
            nc.sync.dma_start(r_d[:], ro[:])
            nc.sync.dma_start(vd_d[:], vd[:])

    nc.compile()
    return nc


def _prepare_inputs(x, W1, b1, W2, b2, K):
    x = np.asarray(x, np.float32)
    W1 = np.asarray(W1, np.float32)
    b1 = np.asarray(b1, np.float32)
    W2 = np.asarray(W2, np.float32)
    b2 = np.asarray(b2, np.float32)
    K = np.asarray(K, np.float32)

    # ---- shared weight prep ----
    w1hi, w1lo = _split_fp16(W1.T)            # [2, 512]
    b1hi, b1lo = _split_fp16(b1)              # [512]
    w1a = np.concatenate(
        [w1hi, w1lo, w1hi, w1lo, b1hi[None, :], b1lo[None, :]], axis=0
    ).astype(FP16)                            # [10, 512]
    w2f = (
        W2.T.reshape(HC, P, H).transpose(1, 0, 2).reshape(P, HC * H).astype(FP16)
    )
    w2b = W2.reshape(HC, P, H).transpose(1, 0, 2).reshape(P, HC * H).astype(FP16)
    w1gv = (
        W1.reshape(HC, P, N_IN).transpose(1, 0, 2).reshape(P, HC * N_IN).astype(FP16)
    )
    b2t = np.ascontiguousarray(b2.reshape(HC, P).T)           # [128, 4] f32
    ho = np.full((P, 1), 0.5, FP16)
    kk = -np.tile(K.reshape(1, 2), (P, 1)).astype(np.float32)  # [128, 2] (-K)

    # ---- per-core inputs ----
    in_maps = []
    for c in range(NCORES):
        xc = x[c * BPC:(c + 1) * BPC]                         # [8192, 2]
        xT = np.ascontiguousarray(xc.T)                       # [2, 8192]
        xhi, xlo = _split_fp16(xT)
        ones = np.ones((1, BPC), FP16)
        xa = np.concatenate([xhi, xhi, xlo, xlo, ones, ones], axis=0).astype(FP16)
        in_maps.append({
            "xa": xa,
            "xc": np.ascontiguousarray(xT.reshape(N_IN, P, TCOLS)),
            "w1a": w1a, "w2f": w2f, "w2b": w2b, "w1gv": w1gv,
            "b2t": b2t, "ho": ho, "kk": kk,
        })
    return in_maps


def kernel(x, W1, b1, W2, b2, K):
    global last_results, last_nc, last_in_maps
    from concourse.bass_utils import run_bass_kernel_spmd

    in_maps = _prepare_inputs(x, W1, b1, W2, b2, K)
    nc = _build_module()
    last_nc, last_in_maps = nc, in_maps
    trace = bool(os.environ.get("KERNEL_TRACE"))
    res = run_bass_kernel_spmd(nc, in_maps, list(range(NCORES)), trace=trace)
    last_results = res

    u = np.concatenate([res.results[c]["u"].reshape(BPC) for c in range(NCORES)])
    r = np.concatenate([res.results[c]["r"].reshape(BPC) for c in range(NCORES)])
    v = np.concatenate([res.results[c]["v"].reshape(BPC) for c in range(NCORES)])
    vd = np.concatenate([res.results[c]["vd"].reshape(BPC) for c in range(NCORES)])

    return (
        u.reshape(B, 1).astype(np.float32),
        r.reshape(B, 1).astype(np.float32),
        v.astype(np.float32),
        vd.reshape(B, 1).astype(np.float32),
    )
